# revision 11
# baseline (speedup 1.0000x reference)
"""Trainium2 Bass kernel for nn_DeTokenizer (EMA detokenizer), packed-int16 I/O.

Computation (forward):
    p_s      = clip(router_probs[0, tok_idx, 1], EPS, 1-EPS)         (M,)
    h_m      = (1-p_m) h_{m-1} + p_m * hidden[m]     (EMA over M chunks, D channels)
    out[t]   = residual[t] + coef[t] * h[j(t)]       j(t) = cumsum(mask)-1
    coef[t]  = mx + (1 - mx)  == 1 in the f32 forward

Strategy: the EMA is linear, so h_m = sum_s exp(LC_m - LC_s) * p_s * hidden[s]
with LC = cumsum(log(1-p)) computed on host in f64. Each of the 8 cores owns
M/8 chunks, processed as blocks of 128: a [128,128] triangular band matrix
(host-built bf16 constant) matmul against the block's hidden tile, plus NW
window matmuls against preceding tiles (older contributions decay below
DECAY_TOL; NW escalates if needed). No collectives: cross-core dependence is
covered by a halo of NW*128 hidden rows.

The problem is HBM-bandwidth bound. Residual and output cross HBM as int8
values sharing one scale S_m per chunk, chosen on host so that
|res| + |h| <= 125*S_m pointwise (using B = abs-EMA bound of |h|). Then
    out_q = res_q + round(h/S)
is an exact integer add: no dequant pass and half the bytes of bf16. To run
the add on DVE at the 2x 16-bit rate (int8 ops are 1x), channel pairs are
packed into int16 lanes: host stages res16 = res_q[2c] + 256*res_q[2c+1];
the device writes round(h*invS) for even/odd channels as strided int8 bytes
into an int16 tile (two ACT ops per block, reading the two PSUM halves of a
d-permuted h), and one broadcast tensor_tensor per block adds res16 + hq16
over all 4 tokens per chunk. Lane sums stay within +-127 by the scale bound,
so no carry crosses a byte boundary except the lo-byte sign borrow, which the
host removes during decode (it knows res_q). DMA per core: mats+hid 2.75 MB
bf16, res 4 MB, out 4 MB -- ~10.75 MB vs ~358 GB/s per-core HBM.
"""

import numpy as np

EPS = 1e-4
N_CORES = 8
P = 128  # SBUF partitions / block size
NMAX = 512  # max matmul free dim (one PSUM bank of f32)
DECAY_TOL = 1e-10

_NC_CACHE: dict = {}


def _build_raw(NB: int, NW: int, D: int, R: int):
    """Raw-Bass build: hand-rolled semaphores, no TileContext.

    TileContext's entry/exit barrier ladders cost ~11 us of a ~50 us
    kernel; the dependency graph here is small and static, so explicit
    sems are worth it.
    """
    from contextlib import ExitStack

    import concourse.bacc as bacc
    import concourse.mybir as mybir

    f32 = mybir.dt.float32
    bf16 = mybir.dt.bfloat16
    i8 = mybir.dt.int8
    u8 = mybir.dt.uint8
    i16 = mybir.dt.int16
    add = mybir.AluOpType.add
    Copy = mybir.ActivationFunctionType.Copy

    C = D // 2
    W1 = NW + 1
    NH = NB + NW
    MC = NB * W1 * P
    LB = R * C

    nc = bacc.Bacc("TRN2", target_bir_lowering=False, debug=False,
                   num_devices=N_CORES)
    head = nc.dram_tensor("head", [P, MC + 2 * D], bf16,
                          kind="ExternalInput").ap()
    hid = nc.dram_tensor("hid", [P, (NH - 2) * D], bf16,
                         kind="ExternalInput").ap()
    res = nc.dram_tensor("res", [P, NB * LB], i16, kind="ExternalInput").ap()
    scl = nc.dram_tensor("scl", [P, NB], f32, kind="ExternalInput").ap()
    out = nc.dram_tensor("out", [P, NB * LB], i16, kind="ExternalOutput").ap()

    nsplit = (D + NMAX - 1) // NMAX
    assert nsplit == 2 and D == 2 * C
    NPS = 4  # PSUM tiles in flight

    ctx = ExitStack()
    with ctx:
        head_t = ctx.enter_context(
            nc.sbuf_tensor("head_t", [P, MC + 2 * D], bf16))
        hid_t = ctx.enter_context(
            nc.sbuf_tensor("hid_t", [P, (NH - 2) * D], bf16))
        res_t = ctx.enter_context(
            nc.sbuf_tensor("res_t", [P, NB * LB], i16))
        scl_t = ctx.enter_context(nc.sbuf_tensor("scl_t", [P, NB], f32))
        hq_ts = [ctx.enter_context(
            nc.sbuf_tensor(f"hq{b}", [P, C], i16)) for b in range(NB)]
        ot_ts = [ctx.enter_context(
            nc.sbuf_tensor(f"ot{b}", [P, LB], i16)) for b in range(NB)]
        ps_ts = [ctx.enter_context(
            nc.psum_tensor(f"ps{j}", [P, D], f32)) for j in range(NPS)]

        s_head = ctx.enter_context(nc.semaphore("s_head"))
        s_hid = ctx.enter_context(nc.semaphore("s_hid"))
        s_scl = ctx.enter_context(nc.semaphore("s_scl"))
        s_res = [ctx.enter_context(nc.semaphore(f"s_res{q}"))
                 for q in range(4)]
        s_mm = ctx.enter_context(nc.semaphore("s_mm"))
        s_hq = ctx.enter_context(nc.semaphore("s_hq"))
        s_tt = ctx.enter_context(nc.semaphore("s_tt"))
        s_ste = ctx.enter_context(nc.semaphore("s_ste"))
        s_sto = ctx.enter_context(nc.semaphore("s_sto"))

        cuts = [min(2, NH), min(5, NH), NH]
        RQ = NB // 4  # blocks per res DMA slice

        def hid_slice(i, c0, c1):
            if i < 2:
                return head_t.ap()[:, MC + i * D + c0:MC + i * D + c1]
            return hid_t.ap()[:, (i - 2) * D + c0:(i - 2) * D + c1]

        def hid_sem_wait(eng, i):
            # wait until hid 128-row tile i is resident
            if i < 2:
                eng.wait_ge(s_head, 16)
            elif i < cuts[1]:
                eng.wait_ge(s_hid, 16)
            else:
                eng.wait_ge(s_hid, 32)

        with nc.Block("k", no_gpsimd_drain=True) as block:
            @block.sync
            def _(sync):
                nc.sync.dma_start(out=head_t.ap()[:], in_=head
                                  ).then_inc(s_head, 16)
                nc.sync.dma_start(
                    out=hid_t.ap()[:, :(cuts[1] - 2) * D],
                    in_=hid[:, :(cuts[1] - 2) * D]).then_inc(s_hid, 16)
                nc.sync.dma_start(
                    out=hid_t.ap()[:, (cuts[1] - 2) * D:],
                    in_=hid[:, (cuts[1] - 2) * D:]).then_inc(s_hid, 16)
                for b in range(0, NB - 2, 2):
                    sync.wait_ge(s_tt, b + 1)
                    nc.sync.dma_start(out=out[:, b * LB:(b + 1) * LB],
                                      in_=ot_ts[b].ap()[:]
                                      ).then_inc(s_ste, 16)
                # final blocks: half-stores on both rings to shrink the
                # post-last-TT drain
                for b in (NB - 2, NB - 1):
                    sync.wait_ge(s_tt, b + 1)
                    nc.sync.dma_start(
                        out=out[:, b * LB:b * LB + LB // 2],
                        in_=ot_ts[b].ap()[:, :LB // 2]).then_inc(s_ste, 16)
                sync.wait_ge(s_ste, 16 * (NB // 2 + 1))
                sync.wait_ge(s_sto, 16 * (NB // 2 + 1))

            @block.scalar
            def _(scalar):
                nc.scalar.dma_start(out=scl_t.ap()[:], in_=scl
                                    ).then_inc(s_scl, 16)
                for q in range(4):
                    nc.scalar.dma_start(
                        out=res_t.ap()[:, q * RQ * LB:(q + 1) * RQ * LB],
                        in_=res[:, q * RQ * LB:(q + 1) * RQ * LB]
                    ).then_inc(s_res[q], 16)
                for b in range(NB):
                    # store for block b-1 (odd blocks): TT b-1 is done well
                    # before ACT b-1 +1 block of scalar work, so no stall.
                    if b >= 1 and (b - 1) % 2 == 1 and b - 1 < NB - 2:
                        scalar.wait_ge(s_tt, b)
                        nc.scalar.dma_start(
                            out=out[:, (b - 1) * LB:b * LB],
                            in_=ot_ts[b - 1].ap()[:]).then_inc(s_sto, 16)
                    scalar.wait_ge(s_mm, b + 1)
                    if b == 0:
                        scalar.wait_ge(s_scl, 16)
                    ps = ps_ts[b % NPS].ap()
                    hqb = hq_ts[b].ap()[:].bitcast(u8).rearrange(
                        "p (c two) -> p two c", two=2)
                    sc_ap = scl_t.ap()[:, b:b + 1]
                    nc.scalar.activation(out=hqb[:, 0].bitcast(i8),
                                         in_=ps[:, 0:C], func=Copy,
                                         scale=sc_ap)
                    nc.scalar.activation(out=hqb[:, 1].bitcast(i8),
                                         in_=ps[:, C:D], func=Copy,
                                         scale=sc_ap).then_inc(s_hq, 1)
                for b in (NB - 2, NB - 1):
                    scalar.wait_ge(s_tt, b + 1)
                    nc.scalar.dma_start(
                        out=out[:, b * LB + LB // 2:(b + 1) * LB],
                        in_=ot_ts[b].ap()[:, LB // 2:]).then_inc(s_sto, 16)

            @block.tensor
            def _(tensor):
                for b in range(NB):
                    hid_sem_wait(tensor, b + NW)
                    if b == 0:
                        tensor.wait_ge(s_head, 16)
                    if b >= NPS:
                        tensor.wait_ge(s_hq, b - NPS + 1)
                    ps = ps_ts[b % NPS].ap()
                    for n in range(nsplit):
                        c0, c1 = n * NMAX, (n + 1) * NMAX
                        for w in range(W1):
                            mm = nc.tensor.matmul(
                                ps[:, c0:c1],
                                lhsT=head_t.ap()[:, (b * W1 + w) * P:
                                                 (b * W1 + w + 1) * P],
                                rhs=hid_slice(b + NW - w, c0, c1),
                                start=(w == 0),
                                stop=(w == NW),
                            )
                            if n == nsplit - 1 and w == NW:
                                mm.then_inc(s_mm, 1)

            @block.vector
            def _(vector):
                for b in range(NB):
                    vector.wait_ge(s_hq, b + 1)
                    vector.wait_ge(s_res[b // RQ], 16)
                    rv = res_t.ap()[:, b * LB:(b + 1) * LB].rearrange(
                        "p (r c) -> p r c", r=R)
                    ov = ot_ts[b].ap()[:].rearrange("p (r c) -> p r c", r=R)
                    hb = hq_ts[b].ap()[:].rearrange(
                        "p (one c) -> p one c", one=1).broadcast_to([P, R, C])
                    nc.vector.tensor_tensor(out=ov, in0=rv, in1=hb, op=add
                                            ).then_inc(s_tt, 1)
        nc.compile()
    return nc


def _build(NB: int, NW: int, D: int, R: int):
    """Build + compile the per-core Bass program (same NEFF for all cores)."""
    import concourse.bacc as bacc
    import concourse.mybir as mybir
    import concourse.tile as tile

    f32 = mybir.dt.float32
    bf16 = mybir.dt.bfloat16
    i8 = mybir.dt.int8
    u8 = mybir.dt.uint8
    i16 = mybir.dt.int16
    add = mybir.AluOpType.add
    Copy = mybir.ActivationFunctionType.Copy

    C = D // 2          # int16 lanes per block column range
    W1 = NW + 1         # band sub-blocks per 128-chunk block
    NH = NB + NW        # hid 128-row tiles
    MC = NB * W1 * P    # mats columns
    LB = R * C          # int16 lanes per block (R tokens x C lanes)

    nc = bacc.Bacc("TRN2", target_bir_lowering=False, debug=False,
                   num_devices=N_CORES)
    # all staging buffers partition-major: [P, ...] with contiguous runs.
    head = nc.dram_tensor("head", [P, MC + 2 * D], bf16,
                          kind="ExternalInput").ap()
    hid = nc.dram_tensor("hid", [P, (NH - 2) * D], bf16,
                         kind="ExternalInput").ap()
    res = nc.dram_tensor("res", [P, NB * LB], i16, kind="ExternalInput").ap()
    scl = nc.dram_tensor("scl", [P, NB], f32, kind="ExternalInput").ap()
    out = nc.dram_tensor("out", [P, NB * LB], i16, kind="ExternalOutput").ap()

    nsplit = (D + NMAX - 1) // NMAX
    assert nsplit == 2 and D == 2 * C

    with tile.TileContext(nc) as tc:
        with tc.tile_pool(name="inp", bufs=1) as mpool, \
             tc.tile_pool(name="psum", bufs=4, space="PSUM") as ppool, \
             tc.tile_pool(name="hqp", bufs=2) as qpool, \
             tc.tile_pool(name="outp", bufs=4) as opool:
        # scalar ring: invS scales then res16 in 2 x 4-block slices
            hpool = mpool
            cpool = mpool
            rpool = mpool
            scl_t = cpool.tile([P, NB], f32)
            nc.scalar.dma_start(out=scl_t[:], in_=scl)
            res_tiles = []
            for q in range(2):
                rt = rpool.tile([P, 4 * LB], i16, tag=f"res{q}")
                nc.scalar.dma_start(
                    out=rt[:], in_=res[:, q * 4 * LB:(q + 1) * 4 * LB])
                res_tiles.append(rt)
            # sync ring: head (mats + hid tiles 0-1), rest of hid in two
            head_t = mpool.tile([P, MC + 2 * D], bf16)
            nc.sync.dma_start(out=head_t[:], in_=head)
            cuts = [min(2, NH), min(5, NH), NH]
            hid_tiles = []
            for ci in range(2):
                c_lo, c_hi = cuts[ci], cuts[ci + 1]
                if c_hi <= c_lo:
                    continue
                t = hpool.tile([P, (c_hi - c_lo) * D], bf16, tag=f"hid{ci}")
                nc.sync.dma_start(
                    out=t[:], in_=hid[:, (c_lo - 2) * D:(c_hi - 2) * D])
                hid_tiles.append((c_lo, c_hi, t))

            def hid_slice(i, c0, c1):
                # hid 128-row tile i, columns [c0, c1)
                if i < 2:
                    return head_t[:, MC + i * D + c0:MC + i * D + c1]
                for c_lo, c_hi, t in hid_tiles:
                    if c_lo <= i < c_hi:
                        return t[:, (i - c_lo) * D + c0:(i - c_lo) * D + c1]
                raise AssertionError(i)

            for b in range(NB):
                ps = ppool.tile([P, D], f32, tag="ps")
                for n in range(nsplit):
                    c0, c1 = n * NMAX, (n + 1) * NMAX
                    for w in range(W1):
                        # w=0: diagonal (triangular) block on own tile;
                        # w>=1: window block on the w-th preceding tile.
                        nc.tensor.matmul(
                            ps[:, c0:c1],
                            lhsT=head_t[:, (b * W1 + w) * P:
                                        (b * W1 + w + 1) * P],
                            rhs=hid_slice(b + NW - w, c0, c1),
                            start=(w == 0),
                            stop=(w == NW),
                        )
                # hq16 lanes: lo byte = i8(round(h_even*invS)) (sign borrow
                # fixed on host), hi byte = i8(round(h_odd*invS)); h columns
                # are d-permuted so evens are PSUM[:, :C], odds PSUM[:, C:].
                hq = qpool.tile([P, C], i16, tag="hq")
                hqb = hq[:].bitcast(u8).rearrange("p (c two) -> p two c", two=2)
                sc_ap = scl_t[:, b:b + 1]
                nc.scalar.activation(out=hqb[:, 0].bitcast(i8), in_=ps[:, 0:C],
                                     func=Copy, scale=sc_ap)
                nc.scalar.activation(out=hqb[:, 1].bitcast(i8), in_=ps[:, C:D],
                                     func=Copy, scale=sc_ap)
                # packed add: out16[p, r, c] = res16[p, r, c] + hq16[p, c]
                q, g = divmod(b, 4)
                ot = opool.tile([P, LB], i16, tag="out")
                rv = res_tiles[q][:, g * LB:(g + 1) * LB].rearrange(
                    "p (r c) -> p r c", r=R)
                ov = ot[:].rearrange("p (r c) -> p r c", r=R)
                hb = hq[:].rearrange("p (one c) -> p one c", one=1
                                     ).broadcast_to([P, R, C])
                nc.vector.tensor_tensor(out=ov, in0=rv, in1=hb, op=add)
                eng = nc.sync if b % 2 == 0 else nc.scalar
                eng.dma_start(out=out[:, b * LB:(b + 1) * LB], in_=ot[:])
    nc.compile()
    return nc


def _host_fallback(hidden_states, residual, token_mask, router_probs):
    """Pure-numpy reference path (off-spec inputs only)."""
    M = hidden_states.shape[1]
    L = residual.shape[1]
    p = router_probs[0, :, 1].astype(np.float64)
    tok_idx = np.nonzero(token_mask[0])[0]
    cp = np.clip(p[tok_idx].astype(np.float32), np.float32(EPS),
                 np.float32(1.0 - EPS)).astype(np.float64)
    h = np.zeros(hidden_states.shape[2], np.float64)
    out_ema = np.empty((M, hidden_states.shape[2]), np.float32)
    hid = hidden_states[0].astype(np.float64)
    for m in range(M):
        h = (1.0 - cp[m]) * h + cp[m] * hid[m]
        out_ema[m] = h.astype(np.float32)
    j = np.clip(np.cumsum(token_mask[0].astype(np.int64)) - 1, 0, M - 1)
    mx = np.max(router_probs[0].astype(np.float32), axis=-1)
    coef = (mx + (np.float32(1.0) - mx)).astype(np.float32)
    out = residual[0].astype(np.float32) + out_ema[j] * coef[:, None]
    return out[None]


def kernel(hidden_states, residual, token_mask, router_probs):
    from concourse import bass_utils
    import ml_dtypes

    bf16 = ml_dtypes.bfloat16

    hidden_states = np.asarray(hidden_states)
    residual = np.asarray(residual)
    token_mask = np.asarray(token_mask)
    router_probs = np.asarray(router_probs)

    _, M, D = hidden_states.shape
    _, L, _ = residual.shape
    R = L // M
    Mc = M // N_CORES      # chunks per core
    Lc = L // N_CORES      # tokens per core
    NB = Mc // P           # 128-chunk blocks per core
    C = D // 2

    mask = token_mask[0]
    mx = np.max(router_probs[0].astype(np.float32), axis=-1)
    coef = (mx + (np.float32(1.0) - mx)).astype(np.float32)
    uniform = (M % (N_CORES * P) == 0 and L % M == 0 and D % 2 == 0
               and np.array_equal(np.flatnonzero(mask), np.arange(M) * R))
    if not uniform or not bool(np.all(coef == np.float32(1.0))):
        return _host_fallback(hidden_states, residual, token_mask,
                              router_probs)

    # ---- host scalar metadata (f64) ----
    p32 = router_probs[0, ::R, 1].astype(np.float32)
    cp32 = np.clip(p32, np.float32(EPS), np.float32(1.0 - EPS))
    cp = cp32.astype(np.float64)
    la = np.log1p(-cp)
    LCx = np.concatenate([[0.0], np.cumsum(la)])  # LCx[i+1] = LC_i

    hid0 = hidden_states[0]
    maxhid = float(np.abs(hid0).max()) or 1.0

    # pick NW: contributions older than NW*P chunks must be < DECAY_TOL
    NW = 1
    while NW < 4:
        g0s = np.arange(NB * N_CORES) * P
        g0s = g0s[g0s - NW * P > 0]
        worst = np.max(np.exp(LCx[g0s] - LCx[g0s - NW * P])) if g0s.size else 0.0
        if worst * maxhid < DECAY_TOL:
            break
        NW += 1
    NH = NB + NW

    # ---- shared scale: S_m >= (|res| + B)/126 pointwise over chunk m ----
    # B = abs-EMA bound: |h_m,d| <= B_m,d = (1-p_m) B_{m-1,d} + p_m |hid_m,d|
    res0 = residual[0]
    abshid = np.abs(hid0).astype(np.float32)
    B = np.empty_like(abshid)
    acc = np.zeros(D, np.float32)
    a32 = (1.0 - cp32).astype(np.float32)
    for m in range(M):
        acc = a32[m] * acc + cp32[m] * abshid[m]
        B[m] = acc
    # /125 (not /127): keeps every int8 lane sum within +-126 even after
    # both roundings, so the packed int16 add stays under 32767 including
    # the +256 lo-byte borrow term (max |v| <= 126+256 + 256*126 = 32638).
    bound = (np.abs(res0).reshape(M, R, D) + B[:, None, :]).max(axis=(1, 2))
    S = np.maximum(bound / 125.0, 1e-30).astype(np.float32)   # (M,)
    invS = (1.0 / S).astype(np.float32)

    # res_q int8 lanes on the shared scale; pack pairs (2c, 2c+1) -> int16
    res_q = np.rint(res0.reshape(M, R, D)
                    / S[:, None, None]).astype(np.int16)
    np.clip(res_q, -127, 127, out=res_q)
    res16 = (res_q[:, :, 0::2] + (res_q[:, :, 1::2] << 8)).astype(np.int16)
    # (M, R, C)

    # d-permutation for hid staging: device h cols = [evens | odds]
    perm = np.concatenate([np.arange(0, D, 2), np.arange(1, D, 2)])
    hid_p = np.ascontiguousarray(hid0[:, perm]).astype(bf16)

    # ---- per-core constants ----
    def band_mats(k):
        m0 = np.zeros((NB * (NW + 1), P, P), np.float32)
        for b in range(NB):
            g0 = k * Mc + b * P
            m_idx = np.arange(g0, g0 + P)
            for w in range(NW + 1):
                s_idx = m_idx - w * P
                valid = s_idx >= 0
                sc = np.where(valid, s_idx, 0)
                blk = (np.exp(LCx[m_idx + 1][None, :] - LCx[sc + 1][:, None])
                       * cp[sc][:, None])
                if w == 0:
                    blk = np.where(s_idx[:, None] <= m_idx[None, :], blk, 0.0)
                blk = np.where(valid[:, None], blk, 0.0)
                m0[b * (NW + 1) + w] = blk.astype(np.float32)
        # partition-major staging: mats_host[p, (f, m)] = m0[f, p, m]
        return np.ascontiguousarray(
            m0.transpose(1, 0, 2).reshape(P, NB * (NW + 1) * P)).astype(bf16)

    in_maps = []
    for k in range(N_CORES):
        lo = k * Mc - NW * P
        if lo < 0:
            halo = np.concatenate(
                [np.zeros((-lo, D), bf16), hid_p[:max(0, k * Mc)]])
        else:
            halo = hid_p[lo:k * Mc]
        hid_k = np.concatenate([halo, hid_p[k * Mc:(k + 1) * Mc]], axis=0)
        # partition-major staging: hid_host[p, (i, d)] = hid_k[i*P + p, d]
        hid_k = np.ascontiguousarray(
            hid_k.reshape(NH, P, D).transpose(1, 0, 2).reshape(P, NH * D))
        # res16 staged [p, (b r c)] for chunk (k*NB + b)*P + p
        r_k = res16[k * Mc:(k + 1) * Mc].reshape(NB, P, R * C)
        r_k = np.ascontiguousarray(
            r_k.transpose(1, 0, 2).reshape(P, NB * R * C))
        s_k = np.ascontiguousarray(
            invS[k * Mc:(k + 1) * Mc].reshape(NB, P).T)
        im = {
            "head": np.ascontiguousarray(
                np.concatenate([band_mats(k), hid_k[:, :2 * D]], axis=1)),
            "hid": np.ascontiguousarray(hid_k[:, 2 * D:]),
            "res": r_k,
            "scl": s_k,
        }
        in_maps.append(im)

    import os
    builder = _build if os.environ.get("DETOK_TILE") else _build_raw
    key = (NB, NW, D, R, builder.__name__)
    if key not in _NC_CACHE:
        _NC_CACHE[key] = builder(*key[:4])
    nc = _NC_CACHE[key]

    results = bass_utils.run_bass_kernel_spmd(
        nc, in_maps, core_ids=list(range(N_CORES)))

    # ---- decode: v = (lo_r + hq_e) + 256*(hi_r + hq_o + [hq_e<0]) ----
    out_full = np.empty((1, L, D), np.float32)
    lo_r = res_q[:, :, 0::2]                       # (M, R, C)
    for k in range(N_CORES):
        o16 = results.results[k]["out"]            # (P, NB*R*C) int16
        o16 = o16.reshape(P, NB, R, C).transpose(1, 0, 2, 3)  # (NB,P,R,C)
        v = o16.astype(np.int32).reshape(Mc, R, C)
        lr = lo_r[k * Mc:(k + 1) * Mc]
        Lq = ((v + 128) & 255) - 128               # lo lane, exact
        Hq = ((v - Lq) >> 8) - (Lq < lr)           # remove sign borrow
        sc = S[k * Mc:(k + 1) * Mc, None, None]
        blk = np.empty((Mc, R, D), np.float32)
        blk[:, :, 0::2] = Lq * sc
        blk[:, :, 1::2] = Hq * sc
        out_full[0, k * Lc:(k + 1) * Lc] = blk.reshape(Lc, D)
    return out_full
